# revision 1
# baseline (speedup 1.0000x reference)
"""Masked (expander) linear layer on 8 Trainium2 NeuronCores.

Computes out = x @ (W * M)^T for
  x: [16384, 2048] f32, W: [2048, 2048] f32, M: [2048, 2048] int32 (0/1)

Sharding: pure data-parallel over rows of x. Each of the 8 cores gets 2048
rows of x plus a replicated (transposed) copy of W and M, computes its
[2048, 2048] output shard entirely locally (mask-multiply on DVE, matmul on
PE), and the host concatenates shards. No collectives.

Device-side design:
 - All tensors are laid out on host so the contraction dim lands on SBUF
   partitions: W and M transposed panel-major ([NT, IN, 512], contiguous
   panels), x transposed per core ([IN, rows]). Layout-only host prep;
   every FLOP of the module (mask multiply + matmul) runs on device. The
   mask is passed as int8 (0/1, lossless repack) to cut DMA traffic.
 - Matmuls run in float32r mode (single-pass fp32_mode=HIGH PE streaming,
   1 cycle/row, vs 4 cycles/row for plain fp32; 1.35e-4 rel err at
   K=2048). The walrus verifier requires f32r operands to come from
   f32r-rounding producers: the DVE mask-multiply writes wm as f32r and
   x streams in through SWDGE cast-DMAs (f32 -> f32r).
 - Each DMA ring carries one stream so per-DMA fixed costs overlap:
   W 1MB k-quarter pieces on the sync HWDGE ring (3-deep staging
   pipeline), masks on the scalar ring, x on the SWDGE ring, outputs on
   the scalar ring. wm is stored as one tile per (n-chunk, k-quarter) so
   matmul sub-groups depend only on their own piece - PE starts ~17us in
   and stays fed through the whole weight load (keeps HAM at full clock).
 - m-tiles are processed in blocks of 4, n-chunk outer inside a block;
   x quarter-tiles are single-buffered and re-streamed just-in-time (the
   next block's quarter q loads right behind this block's last reader of
   quarter q). PSUM groups rotate over 8 banks with evacuation (ScalarE
   copy + DMA) inlined right after each group closes. (PSUM groups
   rotate over all 8 banks.)
"""

from contextlib import ExitStack

import numpy as np

import concourse.bacc as bacc
import concourse.bass as bass
import concourse.mybir as mybir
import concourse.tile as tile
from concourse.bass_utils import run_bass_kernel_spmd

N_CORES = 8
P = 128

FULL_N, FULL_OUT, FULL_IN = 16384, 2048, 2048

MASK_DTYPES = {
    "int8": (mybir.dt.int8, np.int8),
    "int32": (mybir.dt.int32, np.int32),
    "float32": (mybir.dt.float32, np.float32),
}


def build_nc(
    rows: int = FULL_N // N_CORES,
    in_dim: int = FULL_IN,
    out_dim: int = FULL_OUT,
    mm_dtype=mybir.dt.float32r,
    mask_dtype: str = "int8",
    n_chunk: int = 512,
    m_block: int = 4,
):
    """Per-core Bass module: y[rows, out] = x @ (wt * m).

    DRAM layouts: wt/mk panel-major [NT, in_dim, n_chunk]; x transposed
    [in_dim, rows]; y row-major [rows, out_dim].
    """
    assert rows % P == 0 and in_dim % P == 0 and out_dim % n_chunk == 0
    KT = in_dim // P
    MT = rows // P
    NT = out_dim // n_chunk
    assert KT % 4 == 0 and MT % m_block == 0
    KQ = KT // 4
    NB = MT // m_block
    mw = m_block * P  # columns of x per block

    mdt, _ = MASK_DTYPES[mask_dtype]

    nc = bacc.Bacc("TRN2", target_bir_lowering=False, debug=False)
    x = nc.dram_tensor("x", [in_dim, rows], mybir.dt.float32, kind="ExternalInput")
    wt = nc.dram_tensor(
        "wt", [NT, in_dim, n_chunk], mybir.dt.float32, kind="ExternalInput"
    )
    mk = nc.dram_tensor("mk", [NT, in_dim, n_chunk], mdt, kind="ExternalInput")
    y = nc.dram_tensor("y", [rows, out_dim], mybir.dt.float32, kind="ExternalOutput")

    # K-major DRAM views: [.., p, kt, ..]
    wt_v = wt[:, :, :].rearrange("t (kt p) n -> t p kt n", p=P)
    mk_v = mk[:, :, :].rearrange("t (kt p) n -> t p kt n", p=P)
    x_v = x[:, :].rearrange("(kt p) m -> p kt m", p=P)

    with ExitStack() as ctx:
        tc = ctx.enter_context(tile.TileContext(nc))
        wm_pool = ctx.enter_context(tc.tile_pool(name="wm", bufs=1))
        ws_pool = ctx.enter_context(tc.tile_pool(name="ws", bufs=3))
        msk_pool = ctx.enter_context(tc.tile_pool(name="msk", bufs=3))
        xt_pool = ctx.enter_context(tc.tile_pool(name="xt", bufs=1))
        yo_pool = ctx.enter_context(tc.tile_pool(name="yo", bufs=3))
        pm_pool = ctx.enter_context(tc.tile_pool(name="pm", bufs=1, space="PSUM"))

        # Resident masked weight: wm_t[nt][q] of shape [P, KQ, n_chunk]
        wm_t = [
            [
                wm_pool.tile(
                    [P, KQ, n_chunk], mm_dtype, tag=f"wm{nt}_{q}", name=f"wm{nt}_{q}"
                )
                for q in range(4)
            ]
            for nt in range(NT)
        ]
        # x tiles: [P, KQ, m_block*P] per k-quarter (single set; the next
        # block's quarter q streams in right after this block's last reader
        # of quarter q)
        xt_t = [
            xt_pool.tile([P, KQ, mw], mm_dtype, tag=f"xt{q}", name=f"xt{q}")
            for q in range(4)
        ]

        def load_w_piece(nt, q):
            ksl = slice(q * KQ, (q + 1) * KQ)
            # W rides the sync HWDGE ring alone (own per-DMA fixed costs)
            wstage = ws_pool.tile([P, KQ, n_chunk], mybir.dt.float32, tag="ws")
            nc.sync.dma_start(out=wstage[:], in_=wt_v[nt, :, ksl, :])
            # masks ride the scalar ring (done before output stores begin)
            mtile = msk_pool.tile([P, KQ, n_chunk], mdt, tag="mt")
            nc.scalar.dma_start(out=mtile[:], in_=mk_v[nt, :, ksl, :])
            for k in range(KQ):
                # masked multiply; DVE f32r output is the rounding producer
                nc.vector.tensor_mul(
                    wm_t[nt][q][:, k, :], wstage[:, k, :], mtile[:, k, :]
                )

        def load_x_piece(b, q):
            ksl = slice(q * KQ, (q + 1) * KQ)
            # SWDGE cast-DMA f32 -> f32r (the rounding producer); x has the
            # SWDGE ring to itself. Two m-half DMAs: the first half's WAR
            # clears as soon as mb 0/1 finish reading, so the JIT re-stream
            # at block boundaries starts (and lands) earlier.
            hw = mw // 2
            for h in range(2):
                nc.gpsimd.dma_start(
                    out=xt_t[q][:, :, h * hw : (h + 1) * hw],
                    in_=x_v[:, ksl, b * mw + h * hw : b * mw + (h + 1) * hw],
                )

        # ---- prep: x block 0 on the SWDGE ring, W pieces on sync ----
        for q in range(4):
            load_x_piece(0, q)
        for nt in range(NT):
            for q in range(4):
                load_w_piece(nt, q)

        # ---- main: blocks of m_block m-tiles; nt-outer inside a block ----
        for b in range(NB):
            xts = xt_t
            for nt in range(NT):
                # 6 rotating PSUM banks: group g frees its bank 6 groups later
                pms = {
                    mb: pm_pool.tile(
                        [P, n_chunk],
                        mybir.dt.float32,
                        tag=f"pm{(nt * m_block + mb) % 8}",
                        name=f"pm{(nt * m_block + mb) % 8}",
                    )
                    for mb in range(m_block)
                }
                # k-quarter-outer: each sub-group only needs its own pieces
                for q in range(4):
                    for mb in range(m_block):
                        for k in range(KQ):
                            kt = q * KQ + k
                            nc.tensor.matmul(
                                pms[mb][:],
                                xts[q][:, k, bass.ts(mb, P)],
                                wm_t[nt][q][:, k, :],
                                start=(kt == 0),
                                stop=(kt == KT - 1),
                            )
                        if q == 3:
                            # evacuate as soon as this group closes
                            mt = b * m_block + mb
                            yo = yo_pool.tile(
                                [P, n_chunk], mybir.dt.float32, tag="yo"
                            )
                            nc.scalar.copy(yo[:], pms[mb][:])
                            nc.scalar.dma_start(
                                out=y[mt * P : (mt + 1) * P, bass.ts(nt, n_chunk)],
                                in_=yo[:],
                            )
                    if nt == NT - 1 and b + 1 < NB:
                        # last reader of x quarter q just finished; stream in
                        # the next block's quarter q behind it
                        load_x_piece(b + 1, q)

    nc.compile()
    return nc


def _prep_host(input_, weight, mask, mask_dtype="int8", n_chunk=512):
    _, npdt = MASK_DTYPES[mask_dtype]
    in_dim, out_dim = weight.shape[1], weight.shape[0]
    nt = out_dim // n_chunk
    # weight.T -> [NT, IN, n_chunk], each panel contiguous
    wtp = np.ascontiguousarray(weight.T.reshape(in_dim, nt, n_chunk).transpose(1, 0, 2))
    mkp = np.ascontiguousarray(
        mask.T.reshape(in_dim, nt, n_chunk).transpose(1, 0, 2)
    ).astype(npdt)
    rows = input_.shape[0] // N_CORES
    in_maps = []
    for c in range(N_CORES):
        xp = np.ascontiguousarray(input_[c * rows : (c + 1) * rows].T)
        in_maps.append({"x": xp, "wt": wtp, "mk": mkp})
    return in_maps


_CACHE = {}


def _run(input_, weight, mask, trace=False, **build_kw):
    rows_total, in_dim = input_.shape
    out_dim = weight.shape[0]
    key = (rows_total, in_dim, out_dim, tuple(sorted(build_kw.items())))
    if key not in _CACHE:
        _CACHE[key] = build_nc(
            rows=rows_total // N_CORES, in_dim=in_dim, out_dim=out_dim, **build_kw
        )
    nc = _CACHE[key]
    in_maps = _prep_host(
        input_,
        weight,
        mask,
        build_kw.get("mask_dtype", "int8"),
        build_kw.get("n_chunk", 512),
    )
    res = run_bass_kernel_spmd(nc, in_maps, core_ids=list(range(N_CORES)), trace=trace)
    out = np.concatenate([res.results[c]["y"] for c in range(N_CORES)], axis=0)
    return out, res


def kernel(input_, weight, mask):
    input_ = np.asarray(input_, dtype=np.float32)
    weight = np.asarray(weight, dtype=np.float32)
    mask = np.asarray(mask)
    out, _ = _run(input_, weight, mask, trace=False)
    return out



# revision 4
# speedup vs baseline: 1.1609x; 1.1609x over previous
"""Masked (expander) linear layer on 8 Trainium2 NeuronCores.

Computes out = x @ (W * M)^T for
  x: [16384, 2048] f32, W: [2048, 2048] f32, M: [2048, 2048] int32 (0/1)

Sharding: pure data-parallel over rows of x. Each of the 8 cores gets 2048
rows of x plus a replicated (transposed) copy of W and M, computes its
[2048, 2048] output shard entirely locally, and the host concatenates
shards. No collectives.

Device-side design (v2, bf16 all-resident):
 - x and W are repacked to bf16 on the host (same rounding a device-side
   cast-DMA would apply; 1.35e-4 -> 2.1e-3 rel err, far under tolerance),
   the mask to int8. Input HBM traffic per core drops 36MB -> 20MB. The
   mask multiply (the module's elementwise FLOPs) still runs on DVE; all
   matmul FLOPs run on PE.
 - Everything is SBUF-resident: wm (masked weight, 4 n-chunks x 4
   k-quarter tiles, bf16, 64KB/partition) and x (4 m-blocks x 4
   k-quarters, bf16, 64KB/partition). x is loaded exactly once; no JIT
   re-streaming.
 - All input DMAs ride the sync HWDGE ring in exact PE consumption
   order: x block0 quarters interleaved with (mask, W) quarters of
   n-chunk 0, then n-chunks 1-3, then x blocks 1-3. y evacuations ride
   the scalar ring. Arrival therefore tracks the PE's needs: the PE
   starts ~6us in and is paced by the W stream only through the first
   row of (n-chunk, block) pairs.
 - PE order: pairs (nt, b=0) for nt 0..3 first (paced by the W stream),
   then nt-outer over blocks 1..3 (everything resident by then).
 - A short burst of tiny warm-up matmuls on a scratch PSUM bank runs
   while the first DMAs land, so the HAM clock-gate is at full rate
   (2.4 GHz) before the first real matmul issues.
 - PSUM groups rotate over all 8 banks; each group is evacuated
   (ScalarE copy + scalar-ring DMA) right after it closes.
"""

from contextlib import ExitStack

import numpy as np
import ml_dtypes

import concourse.bacc as bacc
import concourse.bass as bass
import concourse.mybir as mybir
import concourse.tile as tile
from concourse.bass_utils import run_bass_kernel_spmd

N_CORES = 8
P = 128

FULL_N, FULL_OUT, FULL_IN = 16384, 2048, 2048


def build_nc(
    rows: int = FULL_N // N_CORES,
    in_dim: int = FULL_IN,
    out_dim: int = FULL_OUT,
    n_chunk: int = 512,
    m_block: int = 4,
    warmup_mms: int = 36,
):
    """Per-core Bass module: y[rows, out] = x @ (wt * m), bf16 inputs.

    DRAM layouts: wt/mk panel-major [NT, in_dim, n_chunk] (wt bf16, mk
    int8); x transposed [in_dim, rows] bf16; y row-major [rows, out_dim]
    f32.
    """
    assert rows % P == 0 and in_dim % P == 0 and out_dim % n_chunk == 0
    KT = in_dim // P
    MT = rows // P
    NT = out_dim // n_chunk
    assert KT % 4 == 0 and MT % m_block == 0
    KQ = KT // 4
    NB = MT // m_block
    mw = m_block * P  # columns of x per block

    bf16 = mybir.dt.bfloat16

    nc = bacc.Bacc("TRN2", target_bir_lowering=False, debug=False)
    x = nc.dram_tensor("x", [in_dim, rows], bf16, kind="ExternalInput")
    wt = nc.dram_tensor("wt", [NT, in_dim, n_chunk], bf16, kind="ExternalInput")
    mk = nc.dram_tensor("mk", [NT, in_dim, n_chunk], mybir.dt.int8, kind="ExternalInput")
    y = nc.dram_tensor("y", [rows, out_dim], mybir.dt.float32, kind="ExternalOutput")

    # K-major DRAM views: [.., p, kt, ..]
    wt_v = wt[:, :, :].rearrange("t (kt p) n -> t p kt n", p=P)
    mk_v = mk[:, :, :].rearrange("t (kt p) n -> t p kt n", p=P)
    x_v = x[:, :].rearrange("(kt p) m -> p kt m", p=P)

    with ExitStack() as ctx:
        tc = ctx.enter_context(tile.TileContext(nc))
        wm_pool = ctx.enter_context(tc.tile_pool(name="wm", bufs=1))
        xt_pool = ctx.enter_context(tc.tile_pool(name="xt", bufs=1))
        ws_pool = ctx.enter_context(tc.tile_pool(name="ws", bufs=4))
        msk_pool = ctx.enter_context(tc.tile_pool(name="msk", bufs=4))
        yo_pool = ctx.enter_context(tc.tile_pool(name="yo", bufs=3))
        wu_pool = ctx.enter_context(tc.tile_pool(name="wu", bufs=1))
        pm_pool = ctx.enter_context(tc.tile_pool(name="pm", bufs=1, space="PSUM"))

        # Resident masked weight: wm_t[nt][q] of shape [P, KQ, n_chunk] bf16
        wm_t = [
            [
                wm_pool.tile([P, KQ, n_chunk], bf16, tag=f"wm{nt}_{q}", name=f"wm{nt}_{q}")
                for q in range(4)
            ]
            for nt in range(NT)
        ]
        # Resident x: xt_t[b][q] of shape [P, KQ, m_block*P] bf16
        xt_t = [
            [
                xt_pool.tile([P, KQ, mw], bf16, tag=f"xt{b}_{q}", name=f"xt{b}_{q}")
                for q in range(4)
            ]
            for b in range(NB)
        ]

        # ---- PE warm-up: tiny matmuls on scratch data keep the HAM
        # activity window busy while the first input DMAs land, so real
        # matmuls start at the full 2.4 GHz clock. Bank 7 is not needed
        # by real groups until pair index 1, long after these drain.
        if warmup_mms:
            wu = wu_pool.tile([P, P], bf16, tag="wu", name="wu")
            nc.vector.memset(wu[:], 0.0)
            pwu = pm_pool.tile([P, 64], mybir.dt.float32, tag="pm7", name="pmwu")
            for i in range(warmup_mms):
                nc.tensor.matmul(pwu[:], wu[:], wu[:, :64], start=True, stop=True)

        def load_x_piece(b, q):
            ksl = slice(q * KQ, (q + 1) * KQ)
            nc.sync.dma_start(
                out=xt_t[b][q][:], in_=x_v[:, ksl, b * mw : (b + 1) * mw]
            )

        def load_w_piece(nt, q):
            ksl = slice(q * KQ, (q + 1) * KQ)
            mtile = msk_pool.tile([P, KQ, n_chunk], mybir.dt.int8, tag="mt")
            nc.sync.dma_start(out=mtile[:], in_=mk_v[nt, :, ksl, :])
            wstage = ws_pool.tile([P, KQ, n_chunk], bf16, tag="ws")
            nc.sync.dma_start(out=wstage[:], in_=wt_v[nt, :, ksl, :])
            # masked multiply on DVE (bf16: 2x throughput), one op per piece
            nc.vector.tensor_mul(wm_t[nt][q][:], wstage[:], mtile[:])

        # ---- input stream, in exact PE consumption order, all on the
        # sync HWDGE ring (FIFO): x0/nt0 interleaved, nt1-3, x1-3.
        for q in range(4):
            load_x_piece(0, q)
            load_w_piece(0, q)
        for nt in range(1, NT):
            for q in range(4):
                load_w_piece(nt, q)
        for b in range(1, NB):
            for q in range(4):
                load_x_piece(b, q)

        # ---- PE: pair (nt, b) = m_block PSUM groups of KT matmuls each
        pair_idx = [0]

        def pair(nt, b):
            g0 = pair_idx[0] * m_block
            pair_idx[0] += 1
            pms = {
                mb: pm_pool.tile(
                    [P, n_chunk],
                    mybir.dt.float32,
                    tag=f"pm{(g0 + mb) % 8}",
                    name=f"pm{(g0 + mb) % 8}",
                )
                for mb in range(m_block)
            }
            for q in range(4):
                for mb in range(m_block):
                    for k in range(KQ):
                        kt = q * KQ + k
                        nc.tensor.matmul(
                            pms[mb][:],
                            xt_t[b][q][:, k, bass.ts(mb, P)],
                            wm_t[nt][q][:, k, :],
                            start=(kt == 0),
                            stop=(kt == KT - 1),
                        )
                    if q == 3:
                        mt = b * m_block + mb
                        yo = yo_pool.tile([P, n_chunk], mybir.dt.float32, tag="yo")
                        nc.scalar.copy(yo[:], pms[mb][:])
                        nc.scalar.dma_start(
                            out=y[mt * P : (mt + 1) * P, bass.ts(nt, n_chunk)],
                            in_=yo[:],
                        )

        # b0 row first (paced by the W stream), then nt-outer over the rest
        for nt in range(NT):
            pair(nt, 0)
        for nt in range(NT):
            for b in range(1, NB):
                pair(nt, b)

    nc.compile()
    return nc


def _prep_host(input_, weight, mask, n_chunk=512):
    bf = ml_dtypes.bfloat16
    in_dim, out_dim = weight.shape[1], weight.shape[0]
    nt = out_dim // n_chunk
    # weight.T -> [NT, IN, n_chunk] bf16, each panel contiguous
    wtp = np.ascontiguousarray(
        weight.T.reshape(in_dim, nt, n_chunk).transpose(1, 0, 2)
    ).astype(bf)
    mkp = np.ascontiguousarray(
        mask.T.reshape(in_dim, nt, n_chunk).transpose(1, 0, 2)
    ).astype(np.int8)
    rows = input_.shape[0] // N_CORES
    in_maps = []
    for c in range(N_CORES):
        xp = input_[c * rows : (c + 1) * rows].T.astype(bf)  # contiguous copy
        in_maps.append({"x": xp, "wt": wtp, "mk": mkp})
    return in_maps


_CACHE = {}


def _run(input_, weight, mask, trace=False, **build_kw):
    rows_total, in_dim = input_.shape
    out_dim = weight.shape[0]
    key = (rows_total, in_dim, out_dim, tuple(sorted(build_kw.items())))
    if key not in _CACHE:
        _CACHE[key] = build_nc(
            rows=rows_total // N_CORES, in_dim=in_dim, out_dim=out_dim, **build_kw
        )
    nc = _CACHE[key]
    in_maps = _prep_host(input_, weight, mask, build_kw.get("n_chunk", 512))
    res = run_bass_kernel_spmd(nc, in_maps, core_ids=list(range(N_CORES)), trace=trace)
    out = np.concatenate([res.results[c]["y"] for c in range(N_CORES)], axis=0)
    return out, res


def kernel(input_, weight, mask):
    input_ = np.asarray(input_, dtype=np.float32)
    weight = np.asarray(weight, dtype=np.float32)
    mask = np.asarray(mask)
    out, _ = _run(input_, weight, mask, trace=False)
    return out
